# revision 78
# baseline (speedup 1.0000x reference)
"""Trainium2 Bass kernel for the DeformationGraph problem.

Math: per batch b and vertex v,
    out[b,v,k] = sum_c W[v,c] * ( sum_d (X[b,v,d]-center[b,c,d]) * R[b,c,k,d]
                                  + center[b,c,k] + V_nodes[b,c,k] )
factors into a vertex-independent per-node affine map:
    t[b,c,k]   = center[b,c,k] + V_nodes[b,c,k] - sum_d center[b,c,d]*R[b,c,k,d]
    out[b,v,k] = sum_d X[b,v,d] * (W @ R[..,k,d])[v]  +  (W @ t[..,k])[v]
i.e. one (V,C)@(C,48) matmul Y = W @ G, then a per-vertex contraction of Y
with [X,1].  W/X/out are sharded over the vertex dim across 8 cores.

Precision: rel-err budget is 2e-2; a single bf16 term everywhere (W, G, x,
the product tensor, and the output) measures ~5e-3 end-to-end.

Per-core pipeline (vertex shard padded to 6272 = 6*1024 + 128):
  - PE, per 1024-vertex pair (G-column layout j = k*16 + d*4 + b, d==3 =
    translation, cols 48:64 zero):
      A_e (K=128, rows 0:64), A_o (K=128, rows 64:128), and the K=32
      B-part as ONE block-diag [64,128] matmul over vertex-pair columns
      (whb[(h*32+c), 512p+j] = W_B[c, 1024p+512h+j]) accumulating into
      the same y tile -> 3 x 512-col streams instead of 4.
  - DVE: one [128,512] tensor_mul  s = y * xd2  (bf16 out to SBUF).
  - PE: the 4-way d-reduction as a 0/1 matmul r[24,n] = RED^T @ s,
    lagged 2 pairs behind so the PE never waits on the DVE.
  - ACT: r (PSUM) -> ro (SBUF, bf16) copies; out DMAs issued from the
    scalar queue right after.
DMA plan: ~3MB/core HBM over three queues (sync/scalar HWDGE + gpsimd
SWDGE).  HWDGE packets round-robin across a queue's outstanding DMAs, so
a chunk's completion tracks the whole queue's drain - the schedule keeps
each queue's early items small and critical.  cst (G/RED constants) rides
as the first CW columns of the xd2 tensor; the xd2 multiply table ships
fully inflated (on-chip replication kept losing the queue-ordering race).
Warmup matmuls use memset-zero weights so the PE clock governor ramps
from ~7.5us with no DMA dependency.
"""

import numpy as np
import ml_dtypes

import concourse.mybir as mybir
import concourse.tile as tile
from concourse import bacc
from concourse.bass_utils import run_bass_kernel_spmd

B, V, C = 4, 50000, 160
N_CORES = 8
VS = V // N_CORES            # 6250 vertices per core
VSP = 6272                   # padded shard: 6 pairs of 1024 + 128 tail
NPAIR = 6
PC = 3200                    # pair-col space: 6*512 + 128
F32 = mybir.dt.float32
BF16 = mybir.dt.bfloat16
NPBF16 = ml_dtypes.bfloat16

CW = 224                     # cst columns prepended to the xc tensor
# wha chunks (vertex cols).  A chunk's completion tracks its queue's
# cumulative drain (packets round-robin), so finer chunks complete
# earlier for the pairs that need them.  First three on scalar's HWDGE
# queue, last two on sync after the xc stream.
WCH = [(0, 2048), (2048, 3072), (3072, 4096), (4096, 5120), (5120, VSP)]
WENG = ["scalar", "scalar", "scalar", "sync", "sync"]
BCH = [(0, 1024), (1024, PC)]  # whb2 DMA chunks (pair cols)
N_WARM = 52                    # N=128 zero-weight ramp matmuls
N_FILL = 4                     # filler matmuls between DMA-gated pairs


def _locate(tiles, chunks, g0, width, offs=None):
    for i, (t, (c0, c1)) in enumerate(zip(tiles, chunks)):
        if c0 <= g0 and g0 + width <= c1:
            off = (offs[i] if offs else 0) + g0 - c0
            return t, slice(off, off + width)
    raise AssertionError(f"col range {g0}+{width} crosses chunk boundary")


def _build_bass():
    nc = bacc.Bacc()

    wha_d = nc.dram_tensor("wha", [128, VSP], BF16, kind="ExternalInput")
    whb_d = nc.dram_tensor("whb", [64, PC], BF16, kind="ExternalInput")
    xc_d = nc.dram_tensor("xc", [128, CW + PC], BF16, kind="ExternalInput")
    outT = nc.dram_tensor("outT", [24, PC], BF16, kind="ExternalOutput")

    with tile.TileContext(nc) as tc:
        with (
            tc.tile_pool(name="cpool", bufs=1) as cpool,
            tc.tile_pool(name="spool", bufs=4) as spool,
            tc.tile_pool(name="ypool", bufs=4, space="PSUM") as ypool,
            tc.tile_pool(name="rpool", bufs=3, space="PSUM") as rpool,
        ):
            # zero-weight PE ramp: no input dependency, starts right after
            # the preamble and keeps the clock governor fed until real
            # work arrives
            wsc = cpool.tile([128, 192], BF16, tag="wsc")
            nc.vector.memset(wsc[:], 0.0)
            ywarm = ypool.tile([64, 128], F32, tag="ywarm", bufs=1)
            for w in range(N_WARM):
                nc.tensor.matmul(ywarm[:], wsc[:, 0:64], wsc[:, 64:192],
                                 start=(w == 0), stop=(w == N_WARM - 1),
                                 skip_group_check=True)

            # --- input DMAs, in priority order per queue ---
            # sync:   xcc0 (cst + xd2 cols 0:1536) | whb0 | xcc1 | wha2
            # scalar: wha0 | wha1  (then copies + out DMAs)
            # gpsimd: whb1
            xcc0 = cpool.tile([128, CW + 1024], BF16, tag="xcc0")
            nc.sync.dma_start(out=xcc0[:], in_=xc_d[:, 0:CW + 1024])

            wha_t = []

            def wha_chunk(i):
                c0, c1 = WCH[i]
                t = cpool.tile([128, c1 - c0], BF16, tag=f"wha{i}")
                eng = nc.scalar if WENG[i] == "scalar" else nc.sync
                eng.dma_start(out=t[:], in_=wha_d[:, c0:c1])
                wha_t.append(t)

            wha_chunk(0)

            whb0 = cpool.tile([64, 1024], BF16, tag="whb0")
            nc.sync.dma_start(out=whb0[:], in_=whb_d[:, 0:1024])
            whb_t = [whb0]

            wha_chunk(1)
            xccm = cpool.tile([128, 1024], BF16, tag="xccm")
            nc.sync.dma_start(out=xccm[:], in_=xc_d[:, CW + 1024:CW + 2048])

            whb1 = cpool.tile([64, PC - 1024], BF16, tag="whb1")
            nc.gpsimd.dma_start(out=whb1[:], in_=whb_d[:, 1024:PC])
            whb_t.append(whb1)

            wha_chunk(2)
            wha_chunk(3)
            xcc1 = cpool.tile([128, PC - 2048], BF16, tag="xcc1")
            nc.sync.dma_start(out=xcc1[:], in_=xc_d[:, CW + 2048:CW + PC])
            wha_chunk(4)

            def xd2_slice(c0, c1, rows=slice(0, 128)):
                if c1 <= 1024:
                    return xcc0[rows, CW + c0:CW + c1]
                if c1 <= 2048:
                    return xccm[rows, c0 - 1024:c1 - 1024]
                return xcc1[rows, c0 - 2048:c1 - 2048]

            ghA = xcc0[:, 0:64]
            RED24 = xcc0[:, 64:88]
            ghB2 = xcc0[0:64, 96:224]  # block-diag [[G_B,0],[0,G_B]]

            ro = cpool.tile([24, PC], BF16, tag="ro")

            def emit_pair(p):
                y = ypool.tile([128, 512], F32, tag="y", bufs=4)
                for h in range(2):
                    g0 = 1024 * p + 512 * h
                    wa, sa = _locate(wha_t, WCH, g0, 512)
                    nc.tensor.matmul(y[64 * h:64 * h + 64, :], ghA,
                                     wa[:, sa], start=True, stop=False,
                                     skip_group_check=True)
                wb, sb = _locate(whb_t, BCH, 512 * p, 512)
                nc.tensor.matmul(y[:], ghB2, wb[:, sb],
                                 start=False, stop=True,
                                 skip_group_check=True)
                s = spool.tile([128, 512], BF16, tag="s")
                nc.vector.tensor_mul(out=s[:], in0=y[:],
                                     in1=xd2_slice(512 * p, 512 * p + 512))
                return s

            def emit_red(p, s, copy_eng="scalar"):
                r = rpool.tile([24, 512], F32, tag="r", bufs=3)
                nc.tensor.matmul(r[:], RED24, s[:], start=True, stop=True,
                                 skip_group_check=True)
                csl = slice(512 * p, 512 * p + 512)
                if copy_eng == "scalar":
                    nc.scalar.copy(out=ro[:, csl], in_=r[:])
                else:
                    nc.vector.tensor_copy(out=ro[:, csl], in_=r[:])
                if p in (1, 3, 5):
                    c0 = 1024 * (p // 2)
                    nc.sync.dma_start(out=outT[:, c0:c0 + 1024],
                                      in_=ro[:, c0:c0 + 1024])

            # software pipeline: RED(p) lags the A/B matmuls by two pairs
            # at first (so the PE never waits on the DVE multiply), then
            # catches up to lag one near the end where the PE is DMA-gated,
            # shortening the serial tail.  RED_AFTER[p] = REDs emitted
            # after pair p's matmuls; all of 0..5 must appear exactly once.
            RED_AFTER = {2: [0], 3: [1], 4: [2, 3], 5: [4]}
            assert sorted(r for v in RED_AFTER.values() for r in v) == [
                0, 1, 2, 3, 4]
            def pe_filler(n):
                # cheap zero-weight matmuls: keep the PE busy through
                # DMA-gated gaps so the clock governor stays ramped
                for w in range(n):
                    nc.tensor.matmul(ywarm[:], wsc[:, 0:64], wsc[:, 64:192],
                                     start=True, stop=True,
                                     skip_group_check=True)

            s_tiles = {}
            for p in range(NPAIR):
                s_tiles[p] = emit_pair(p)
                for rp in RED_AFTER.get(p, []):
                    emit_red(rp, s_tiles.pop(rp))
                if p < 4:
                    pe_filler(N_FILL)

            # 128-vertex tail (single half), using pooled tile slices
            yt = ypool.tile([128, 512], F32, tag="y", bufs=4)
            wa, sa = _locate(wha_t, WCH, 6144, 128)
            wb, sb = _locate(whb_t, BCH, 3072, 128)
            nc.tensor.matmul(yt[0:64, 0:128], ghA, wa[:, sa],
                             start=True, stop=False, skip_group_check=True)
            nc.tensor.matmul(yt[0:64, 0:128], ghB2[:, 0:64], wb[:, sb],
                             start=False, stop=True, skip_group_check=True)
            st = spool.tile([128, 512], BF16, tag="s")
            nc.vector.tensor_mul(out=st[0:64, 0:128], in0=yt[0:64, 0:128],
                                 in1=xd2_slice(3072, 3200, slice(0, 64)))

            emit_red(5, s_tiles.pop(5), copy_eng="vector")
            rt = rpool.tile([24, 512], F32, tag="r", bufs=3)
            nc.tensor.matmul(rt[:, 0:128], xcc0[0:64, 64:88],
                             st[0:64, 0:128],
                             start=True, stop=True, skip_group_check=True)
            nc.scalar.copy(out=ro[:, 3072:3200], in_=rt[:, 0:128])
            nc.sync.dma_start(out=outT[:, 3072:3200], in_=ro[:, 3072:3200])
    nc.finalize()
    return nc


_NC_CACHE = None


def _get_nc():
    global _NC_CACHE
    if _NC_CACHE is None:
        _NC_CACHE = _build_bass()
    return _NC_CACHE


def _host_prep(X, V_nodes, rot6d_nodes, W_nodes, idx_nn_to_nodes):
    """Small per-node math (B*C=640 rows) + shard/layout of the big tensors."""
    X = np.asarray(X, np.float32)
    Vn = np.asarray(V_nodes, np.float32)
    d6 = np.asarray(rot6d_nodes, np.float32)
    W = np.asarray(W_nodes, np.float32)
    idx = np.asarray(idx_nn_to_nodes).astype(np.int64)

    a1, a2 = d6[..., :3], d6[..., 3:]
    eps = np.float32(1e-8)
    n1 = np.sqrt(np.sum(a1 * a1, -1, keepdims=True, dtype=np.float32))
    b1 = a1 / np.maximum(n1, eps)
    dot = np.sum(b1 * a2, -1, keepdims=True, dtype=np.float32)
    a2p = a2 - dot * b1
    n2 = np.sqrt(np.sum(a2p * a2p, -1, keepdims=True, dtype=np.float32))
    b2 = a2p / np.maximum(n2, eps)
    b3 = np.cross(b1, b2)
    R = np.stack([b1, b2, b3], axis=-2).astype(np.float32)  # (B,C,3,3) [b,c,k,d]

    center = X[:, idx, :]                                   # (B,C,3)
    t = (center + Vn - np.einsum('bcd,bckd->bck', center, R)).astype(np.float32)

    # G columns at j = k*16 + d*4 + b (d==3 = translation); cols 48:64 zero
    Gv = np.zeros((C, 4, 4, 4), np.float32)
    Gv[:, 0:3, 0:3, :] = np.transpose(R, (1, 2, 3, 0))
    Gv[:, 0:3, 3, :] = np.transpose(t, (1, 2, 0))
    G = Gv.reshape(C, 64)

    RED = np.zeros((2, 4, 4, 4, 24), np.float32)
    for h in range(2):
        for k in range(3):
            for b in range(B):
                RED[h, k, :, b, h * 12 + k * 4 + b] = 1.0
    RED = RED.reshape(128, 24)

    cst = np.zeros((128, CW), NPBF16)
    cst[:, 0:64] = G[0:128].astype(NPBF16)
    cst[:, 64:88] = RED.astype(NPBF16)
    gB = G[128:160].astype(NPBF16)             # [32, 64]
    cst[0:32, 96:160] = gB                     # even-half block
    cst[32:64, 160:224] = gB                   # odd-half block

    Wb = W.astype(NPBF16)
    in_maps = []
    for i in range(N_CORES):
        vsl = slice(i * VS, (i + 1) * VS)
        wt = np.zeros((160, VSP), NPBF16)
        wt[:, :VS] = Wb[vsl].T
        wha = np.ascontiguousarray(wt[0:128])
        # B part in vertex-pair-column layout: whb[(h*32+c), 512p+j] =
        # W_B[c, 1024p + 512h + j]; tail (cols 3072:3200) even-half only
        bp = wt[128:160]                       # [32, VSP]
        whb = np.zeros((64, PC), NPBF16)
        whb[:, 0:3072] = bp[:, :6144].reshape(32, NPAIR, 2, 512).transpose(
            2, 0, 1, 3).reshape(64, 3072)
        whb[0:32, 3072:3200] = bp[:, 6144:6272]

        Xs = np.zeros((B, VSP, 3), np.float32)
        Xs[:, :VS] = X[:, vsl, :]
        xc = np.zeros((2, 4, 4, PC), np.float32)        # [h, d, b, col]
        main = Xs[:, :6144].reshape(B, NPAIR, 2, 512, 3)
        xc[:, 0:3, :, 0:3072] = np.transpose(
            main, (2, 4, 0, 1, 3)).reshape(2, 3, B, 3072)
        xc[:, 3, :, 0:3072] = 1.0
        xc[0, 0:3, :, 3072:3200] = np.transpose(Xs[:, 6144:6272], (2, 0, 1))
        xc[0, 3, :, 3072:3200] = 1.0
        xc = xc.reshape(2, 16, PC)
        # cst + fully inflated xd2 (4 copies per half) as one tensor
        xcc = np.zeros((128, CW + PC), NPBF16)
        xcc[:, 0:CW] = cst
        xcc[:, CW:] = np.concatenate(
            [xc[0]] * 4 + [xc[1]] * 4, 0).astype(NPBF16)

        in_maps.append({"wha": wha, "whb": whb, "xc": xcc})
    return in_maps


def _gather(results):
    out = np.empty((B, V, 3), np.float32)
    for i, res in enumerate(results):
        o = res["outT"].astype(np.float32).reshape(2, 3, 4, PC)  # [h,k,b,col]
        om = o[:, :, :, 0:3072].reshape(2, 3, 4, NPAIR, 512)
        block = np.empty((B, VSP, 3), np.float32)
        block[:, :6144] = np.transpose(om, (2, 3, 0, 4, 1)).reshape(B, 6144, 3)
        block[:, 6144:6272] = np.transpose(o[0, :, :, 3072:3200], (1, 2, 0))
        out[:, i * VS:(i + 1) * VS] = block[:, :VS]
    return out


def kernel(X, V_nodes, rot6d_nodes, W_nodes, idx_nn_to_nodes, **run_kwargs):
    in_maps = _host_prep(X, V_nodes, rot6d_nodes, W_nodes, idx_nn_to_nodes)
    res = run_bass_kernel_spmd(_get_nc(), in_maps,
                               core_ids=list(range(N_CORES)), **run_kwargs)
    out = _gather(res.results)
    kernel.last_run = res
    return out
